# revision 25
# baseline (speedup 1.0000x reference)
"""Multi-head causal attention (B=4, S=2048, D=1024, H=16, HD=64) on 8 TRN2 cores.

Strategy:
  - Head-parallel: core i computes heads {2i, 2i+1} for all tokens.
    Host pre-transposes x -> xT [D, B*S], folds the 1/sqrt(HD) scale into Wq,
    converts matmul inputs to bf16, and adds bo at the end.
  - On device per core and per batch: qT/kT/vT projections (weights
    stationary, xT moving), scores computed transposed [k, q] with the two
    heads packed via PE row tiling (K=64 each), exp on ACT, PV matmul with
    stationary [v | 1] so the softmax denominator lands in output row 64,
    normalize via fast reciprocal + gpsimd partition_broadcast.
  - Software-pipelined flat slot schedule: scores/exp run LAG slots ahead of
    the PV consumer so the ACT exp latency never stalls the PE; the next
    batch's projection matmuls and the v transposes are woven between slots
    as PE filler; all output projections are deferred to an epilogue that
    runs under the final AllToAll's latency.
  - One AllToAll per (batch, half) reshards head-outputs feature-major.
"""

import sys

sys.path.insert(0, "/opt/trn_rl_repo")

import numpy as np

import concourse.bass as bass
import concourse.mybir as mybir
import concourse.tile as tile
from concourse import bacc, bass_utils

FP = mybir.dt.float32
BF = mybir.dt.bfloat16
AOP = mybir.AluOpType
AFT = mybir.ActivationFunctionType

B, S, D, H = 4, 2048, 1024, 16
HD = 64
N_CORES = 8
NT = B * S  # 8192 tokens
TOK_PER_CORE = NT // N_CORES  # 1024
KD = D // 128  # 8 contraction tiles for the projections
LAG = 3  # scores/exp lookahead (slots) ahead of PV


def build_nc():
    nc = bacc.Bacc(None, target_bir_lowering=False, debug=False, num_devices=N_CORES)

    xt = nc.dram_tensor("xt", [16, 128, KD, 512], BF, kind="ExternalInput")
    wqk = nc.dram_tensor("wqk", [128, 2 * KD, 128], BF, kind="ExternalInput")
    wv = nc.dram_tensor("wv", [128, KD, 128], BF, kind="ExternalInput")
    bqk = nc.dram_tensor("bqk", [2, 128, 1], FP, kind="ExternalInput")
    bvb = nc.dram_tensor("bv", [128, 1], FP, kind="ExternalInput")
    wo = nc.dram_tensor("wo", [128, KD, D], BF, kind="ExternalInput")
    maskd = nc.dram_tensor("mask", [128, 896], BF, kind="ExternalInput")
    identd = nc.dram_tensor("ident", [128, 128], BF, kind="ExternalInput")
    out = nc.dram_tensor("out", [TOK_PER_CORE, D], FP, kind="ExternalOutput")

    with tile.TileContext(nc) as tc:
        with (
            tc.tile_pool(name="const", bufs=1) as const,
            tc.tile_pool(name="xtp", bufs=5) as xtp,
            tc.tile_pool(name="qkv", bufs=2) as qkv,
            tc.tile_pool(name="vnp", bufs=18) as vnp,
            tc.tile_pool(name="esp", bufs=6) as esp,
            tc.tile_pool(name="small", bufs=4) as small,
            tc.tile_pool(name="onp", bufs=6) as onp,
            tc.tile_pool(name="actp", bufs=4) as actp,
            tc.tile_pool(name="oop", bufs=3) as oop,
            tc.tile_pool(name="ps_mm", bufs=2, space="PSUM") as ps_mm,
            tc.tile_pool(name="ps_s", bufs=2, space="PSUM") as ps_s,
            tc.tile_pool(name="ps_o", bufs=1, space="PSUM") as ps_o,
            tc.tile_pool(name="dram", bufs=1, space="DRAM") as dram,
        ):
            # merged per-batch exchange buffers [rank, half, feat, tok]; batch
            # B-1 keeps per-half buffers so the tail exchange stays small
            cc_ins = [
                dram.tile([N_CORES, 2, 128, 128], BF, name=f"cc_in{b}")
                for b in range(B - 1)
            ] + [
                [
                    dram.tile([N_CORES, 128, 128], BF, name=f"cc_in3_{hf}")
                    for hf in range(2)
                ]
            ]
            cc_outs = [
                dram.tile([N_CORES, 2, 128, 128], BF, name=f"cc_out{b}")
                for b in range(B - 1)
            ] + [
                [
                    dram.tile([N_CORES, 128, 128], BF, name=f"cc_out3_{hf}")
                    for hf in range(2)
                ]
            ]

            # ---- resident constants (interleaved with the first xt slabs so
            # the prologue projections are never DMA-gated) ----
            wqk_sb = const.tile([128, 2 * KD, 128], BF, name="wqk_sb")
            nc.scalar.dma_start(wqk_sb[:], wqk[:])
            wv_sb = const.tile([128, KD, 128], BF, name="wv_sb")
            mask_sb = const.tile([128, 896], BF, name="mask_sb")
            ident_sb = const.tile([128, 128], BF, name="ident_sb")
            bqk_sb = const.tile([128, 2], FP, name="bqk_sb")
            bv_sb = const.tile([128, 1], FP, name="bv_sb")
            wo_sb = const.tile([128, KD, D], BF, name="wo_sb")

            def load_consts_early():
                nc.scalar.dma_start(wv_sb[:], wv[:])
                nc.scalar.dma_start(ident_sb[:], identd[:])
                nc.scalar.dma_start(
                    bqk_sb[:], bqk.rearrange("h p one -> p (h one)")
                )
                nc.scalar.dma_start(bv_sb[:], bvb[:])

            def load_consts_late():
                nc.scalar.dma_start(mask_sb[:], maskd[:])

            at_tiles = {}

            def oproj_units(bb, pos):
                """Output projection for token tile (batch bb, half pos) as
                two filler units of 8 matmuls each."""

                def unit(nn):
                    def emit():
                        if nn == 0:
                            at = actp.tile(
                                [128, N_CORES, 128], BF, name="at", tag="at"
                            )
                            if bb < B - 1:
                                src = cc_outs[bb][:, pos, :, :]
                            else:
                                src = cc_outs[bb][pos][:]
                            nc.sync.dma_start(at[:], src.rearrange("f p t -> p f t"))
                            at_tiles[(bb, pos)] = at
                        at = at_tiles[(bb, pos)]
                        row0 = (2 * bb + pos) * 128
                        ps = ps_o.tile([128, 512], FP, name="ps_op", tag=f"o{nn}")
                        for ft in range(N_CORES):
                            nc.tensor.matmul(
                                ps[:],
                                lhsT=at[:, ft, :],
                                rhs=wo_sb[:, ft, nn * 512 : (nn + 1) * 512],
                                start=(ft == 0),
                                stop=(ft == N_CORES - 1),
                            )
                        oo = oop.tile([128, 512], FP, name="oo", tag="oo")
                        nc.vector.tensor_copy(out=oo[:], in_=ps[:])
                        nc.sync.dma_start(
                            out[row0 : row0 + 128, nn * 512 : (nn + 1) * 512], oo[:]
                        )

                    return emit

                return [unit(0), unit(1)]

            def emit_oproj(bb, pos):
                for u in oproj_units(bb, pos):
                    u()

            def emit_a2a(bb, hf=None):
                if bb < B - 1:
                    ins, outs = cc_ins[bb][:], cc_outs[bb][:]
                else:
                    ins, outs = cc_ins[bb][hf][:], cc_outs[bb][hf][:]
                nc.gpsimd.collective_compute(
                    "AllToAll",
                    AOP.bypass,
                    replica_groups=[list(range(N_CORES))],
                    ins=[ins.opt()],
                    outs=[outs.opt()],
                )

            qkv_tiles = {}

            def alloc_qkv(b):
                qkv_tiles[b] = (
                    qkv.tile([128, S], BF, name="qT", tag="qT"),
                    qkv.tile([128, S], BF, name="kT", tag="kT"),
                    qkv.tile([128, S], BF, name="vT", tag="vT"),
                )

            # ---- projection of batch b, slab st, emitted as 6 filler units
            # (qk_h0 first half / second half, qk_h1 halves, v halves) ----
            xt_tiles = {}

            def load_xt(b, st):
                xt_st = xtp.tile([128, KD, 512], BF, name="xt_st", tag="xt")
                nc.scalar.dma_start(xt_st[:], xt[4 * b + st])
                xt_tiles[(b, st)] = xt_st

            def proj_units(b, st):
                """Return a list of closures, each emitting ~4 proj matmuls."""
                units = []

                ps_holder = {}

                def qk_half(h, half):
                    qT, kT, _ = qkv_tiles[b]
                    xts = xt_tiles[(b, st)]
                    if half == 0:
                        ps_holder[h] = ps_mm.tile([128, 512], FP, name="ps_qk", tag="mm")
                    ps = ps_holder[h]
                    for kd in range(4 * half, 4 * half + 4):
                        nc.tensor.matmul(
                            ps[:],
                            lhsT=wqk_sb[:, h * KD + kd, :],
                            rhs=xts[:, kd, :],
                            start=(kd == 0),
                            stop=(kd == KD - 1),
                        )
                    if half == 1:
                        nc.vector.tensor_scalar(
                            qT[h * 64 : h * 64 + 64, st * 512 : (st + 1) * 512],
                            ps[0:64, :],
                            bqk_sb[0:64, h : h + 1],
                            None,
                            AOP.add,
                        )
                        nc.vector.tensor_scalar(
                            kT[h * 64 : h * 64 + 64, st * 512 : (st + 1) * 512],
                            ps[64:128, :],
                            bqk_sb[64:128, h : h + 1],
                            None,
                            AOP.add,
                        )

                def v_half(half):
                    _, _, vT = qkv_tiles[b]
                    xts = xt_tiles[(b, st)]
                    if half == 0:
                        ps_holder["v"] = ps_mm.tile([128, 512], FP, name="ps_v", tag="mm")
                    ps = ps_holder["v"]
                    for kd in range(4 * half, 4 * half + 4):
                        nc.tensor.matmul(
                            ps[:],
                            lhsT=wv_sb[:, kd, :],
                            rhs=xts[:, kd, :],
                            start=(kd == 0),
                            stop=(kd == KD - 1),
                        )
                    if half == 1:
                        nc.vector.tensor_scalar(
                            vT[:, st * 512 : (st + 1) * 512],
                            ps[:],
                            bv_sb[:, 0:1],
                            None,
                            AOP.add,
                        )

                units.append(lambda: qk_half(0, 0))
                units.append(lambda: qk_half(0, 1))
                units.append(lambda: qk_half(1, 0))
                units.append(lambda: qk_half(1, 1))
                units.append(lambda: v_half(0))
                units.append(lambda: v_half(1))
                return units

            vn_tiles = {}

            def vtrans_unit(b, kc):
                # vT tile kc -> v natural [token, hd] tile with ones columns
                def emit():
                    _, _, vT = qkv_tiles[b]
                    pst = ps_mm.tile([128, 128], BF, name="ps_t", tag="mm")
                    nc.tensor.transpose(
                        pst[:], vT[:, kc * 128 : (kc + 1) * 128], ident_sb[:]
                    )
                    vn = vnp.tile([128, 130], BF, name="vn", tag="vn")
                    nc.vector.tensor_copy(
                        out=vn[:].rearrange("p (h c) -> p h c", c=65)[:, :, 0:64],
                        in_=pst[:].rearrange("p (h d) -> p h d", d=64),
                    )
                    nc.vector.tensor_copy(
                        out=vn[:].rearrange("p (h c) -> p h c", c=65)[:, :, 64:65],
                        in_=mask_sb[:, 894:896].rearrange("p (h c) -> p h c", c=1),
                    )
                    vn_tiles[(b, kc)] = vn

                return emit

            es_tiles = {}

            def emit_scores_exp(b, qi, ki):
                qT, kT, _ = qkv_tiles[b]
                j = ki - 4 * qi  # >= 0 on diagonal tiles
                c0 = 128 * max(j, 0)  # first useful column of this q-tile
                pss = ps_s.tile([128, 1024], FP, name="ps_sc", tag="sc")
                es = esp.tile([128, 1024], BF, name="es", tag="es")
                for h in range(2):
                    nc.tensor.matmul(
                        pss[:, h * 512 + c0 : (h + 1) * 512],
                        lhsT=kT[h * 64 : h * 64 + 64, ki * 128 : (ki + 1) * 128],
                        rhs=qT[h * 64 : h * 64 + 64, qi * 512 + c0 : (qi + 1) * 512],
                        start=True,
                        stop=True,
                        tile_position=(h * 64, 0),
                    )
                if c0 >= 256:
                    for h in range(2):
                        nc.scalar.activation(
                            es[:, h * 512 + c0 : (h + 1) * 512],
                            pss[:, h * 512 + c0 : (h + 1) * 512],
                            AFT.Exp,
                        )
                else:
                    nc.scalar.activation(es[:], pss[:], AFT.Exp)
                if j >= 0:
                    # only the 128-wide boundary block straddles the
                    # diagonal; columns beyond it are fully allowed
                    for h in range(2):
                        nc.vector.tensor_tensor(
                            es[:, h * 512 + c0 : h * 512 + c0 + 128],
                            es[:, h * 512 + c0 : h * 512 + c0 + 128],
                            mask_sb[:, 384:512],
                            AOP.mult,
                        )
                es_tiles[(b, qi, ki)] = es

            po_tiles = {}

            def emit_pv(b, qi, ki):
                j = ki - 4 * qi
                c0 = 128 * max(j, 0)
                nki = 4 * (qi + 1)
                if ki == 0:
                    po_tiles[qi] = [
                        ps_o.tile([65, 512], FP, name=f"po{h}", tag=f"o{h}")
                        for h in range(2)
                    ]
                po = po_tiles[qi]
                es = es_tiles.pop((b, qi, ki))
                for h in range(2):
                    nc.tensor.matmul(
                        po[h][:, c0:512],
                        lhsT=vn_tiles[(b, ki)][:, h * 65 : (h + 1) * 65],
                        rhs=es[:, h * 512 + c0 : (h + 1) * 512],
                        start=(ki == 0),
                        stop=(ki == nki - 1),
                    )

            def emit_normalize(b, qi):
                # drain po, normalize and scatter into this batch's A2A buffer
                po = po_tiles.pop(qi)
                t0r = (4 * qi) % 8  # first destination rank of this q-tile
                # for the last half-batch the broadcast runs on the PE (idle
                # there) instead of gpsimd, keeping the final exchange off the
                # gpsimd queue's latency
                fast = b == B - 1 and qi >= 2
                for h in range(2):
                    oc = small.tile([65, 512], FP, name="oc", tag="oc")
                    nc.vector.tensor_copy(out=oc[:], in_=po[h][:])
                    den = small.tile([1, 512], FP, name="den", tag="den")
                    nc.vector.tensor_copy(out=den[:], in_=oc[64:65, :])
                    if fast:
                        bcp = ps_mm.tile([64, 512], FP, name="bcp", tag="mm")
                        nc.tensor.matmul(
                            bcp[:],
                            lhsT=ones64[:],
                            rhs=den[0:1, :],
                            start=True,
                            stop=True,
                        )
                        bc_ap = bcp[:]
                    else:
                        bc = small.tile([64, 512], FP, name="bc", tag="bc")
                        nc.gpsimd.partition_broadcast(bc[:], den[0:1, :], channels=64)
                        bc_ap = bc[:]
                    rc = small.tile([64, 512], FP, name="rc", tag="rc")
                    nc.vector.reciprocal_approx_fast(out=rc[:], in_=bc_ap)
                    on = onp.tile([64, 512], BF, name="on", tag="on")
                    nc.vector.tensor_tensor(on[:], oc[0:64, :], rc[:], AOP.mult)
                    if b < B - 1:
                        dst = cc_ins[b][
                            t0r : t0r + 4, qi // 2, h * 64 : (h + 1) * 64, :
                        ]
                    else:
                        dst = cc_ins[b][qi // 2][
                            t0r : t0r + 4, h * 64 : (h + 1) * 64, :
                        ]
                    nc.sync.dma_start(
                        dst.rearrange("r p t -> p r t"),
                        on[:].rearrange("p (r t) -> p r t", r=4),
                    )

            # ---- software-pipelined schedule ----
            # PE warmup on junk data while the first DMAs land (HAM ramp)
            warm = const.tile([128, 640], BF, name="warm")
            nc.gpsimd.memset(warm[:], 0.25)
            ones64 = const.tile([1, 64], FP, name="ones64")
            nc.gpsimd.memset(ones64[:], 1.0)
            wps = ps_mm.tile([128, 512], FP, name="wps", tag="mm")
            for w in range(24):
                nc.tensor.matmul(
                    wps[:],
                    lhsT=warm[:, 0:128],
                    rhs=warm[:, 128:640],
                    start=(w == 0),
                    stop=(w == 23),
                )

            # prologue: batch 0 projections as a straight blob
            alloc_qkv(0)
            load_xt(0, 0)
            load_consts_early()
            load_xt(0, 1)
            load_xt(0, 2)
            load_xt(0, 3)
            load_consts_late()
            for st in range(4):
                for u in proj_units(0, st):
                    u()
            nc.scalar.dma_start(wo_sb[:], wo[:])

            # one continuous slot stream across all batches; the LAG and the
            # filler queues cross batch boundaries so the PE never drains
            pairs = [(qi, ki) for qi in range(4) for ki in range(4 * (qi + 1))]
            NP = len(pairs)  # 40
            gpairs = [(b, qi, ki) for b in range(B) for (qi, ki) in pairs]
            NG = len(gpairs)  # 160

            # oproj tiles woven as fillers once their exchange has completed:
            # batch 3's slots are ACT-bound with no next-batch proj work, so
            # they absorb batch 3's own last proj slab and early oproj tiles
            oproj_fill_at = {3: [(0, 0), (0, 1), (1, 0)]}

            vt_fill = []
            ms_fill = []
            for s in range(NG + LAG):
                if s < NG and s % NP == 0:
                    b = s // NP
                    # batch 3's last two v-transpose tiles depend on its own
                    # deferred proj slab; queue them behind it in misc order
                    n_vt = 12 if b == B - 1 else S // 128
                    vt_fill.extend(vtrans_unit(b, kc) for kc in range(n_vt))
                    if b + 1 < B:
                        alloc_qkv(b + 1)
                        for st in range(4):
                            load_xt(b + 1, st)
                        n_st = 3 if b + 1 == B - 1 else 4
                        for st in range(n_st):
                            ms_fill.extend(proj_units(b + 1, st))
                    if b == B - 1:
                        ms_fill.extend(proj_units(b, 3))
                        ms_fill.extend(
                            vtrans_unit(b, kc) for kc in range(12, S // 128)
                        )
                    for (ob, opos) in oproj_fill_at.get(b, []):
                        ms_fill.extend(oproj_units(ob, opos))
                if s < NG:
                    b, qi, ki = gpairs[s]
                    emit_scores_exp(b, qi, ki)
                # fillers: vtrans at 2/slot, then misc units at 1/slot
                for _ in range(2):
                    if vt_fill:
                        vt_fill.pop(0)()
                if not vt_fill and ms_fill:
                    ms_fill.pop(0)()
                if s % NP == LAG and not vt_fill and ms_fill:
                    # extra filler covers the po-ring handoff at batch starts
                    ms_fill.pop(0)()
                if s >= LAG:
                    b, qi, ki = gpairs[s - LAG]
                    emit_pv(b, qi, ki)
                    if ki == 4 * (qi + 1) - 1:
                        emit_normalize(b, qi)
                        if b < B - 1:
                            if qi == 3:
                                emit_a2a(b)
                        else:
                            if qi == 1:
                                emit_a2a(b, 0)
                            if qi == 3:
                                emit_a2a(b, 1)
            for u in vt_fill + ms_fill:
                u()

            # epilogue: remaining output projections; the ones not depending
            # on the final AllToAll run under its latency
            for bb, pos in [(1, 1), (2, 0), (2, 1), (3, 0)]:
                emit_oproj(bb, pos)
            # keep the PE p-state hot while the final exchange lands
            wps2 = ps_mm.tile([128, 512], FP, name="wps2", tag="mm")
            for w in range(12):
                nc.tensor.matmul(
                    wps2[:],
                    lhsT=warm[:, 0:128],
                    rhs=warm[:, 128:640],
                    start=(w == 0),
                    stop=(w == 11),
                )
            emit_oproj(B - 1, 1)

    nc.finalize()
    return nc


_NC_CACHE = None


def _get_nc():
    global _NC_CACHE
    if _NC_CACHE is None:
        _NC_CACHE = build_nc()
    return _NC_CACHE


def make_in_maps(x, Wqkv, bqkv, Wo):
    import ml_dtypes

    bf16 = ml_dtypes.bfloat16
    scale = HD ** -0.5
    xT = x.reshape(NT, D).T.astype(bf16)  # [D, NT]
    xtn = np.ascontiguousarray(
        xT.reshape(KD, 128, 16, 512).transpose(2, 1, 0, 3)
    )  # [slab, p, kd, t]
    mask = (np.arange(896)[None, :] - 384 >= np.arange(128)[:, None]).astype(bf16)
    ident = np.eye(128, dtype=np.float32).astype(bf16)
    wo = np.ascontiguousarray(Wo.astype(bf16).reshape(KD, 128, D).transpose(1, 0, 2))
    in_maps = []
    for c in range(N_CORES):
        h0, h1 = 2 * c, 2 * c + 1
        wqk_c = np.stack(
            [
                np.concatenate(
                    [Wqkv[h][:, 0:64] * scale, Wqkv[h][:, 64:128]], axis=1
                )
                for h in (h0, h1)
            ]
        ).astype(bf16)
        wqk_c = (
            wqk_c.reshape(2, KD, 128, 128).transpose(2, 0, 1, 3).reshape(128, 2 * KD, 128)
        )
        wv_c = np.concatenate(
            [Wqkv[h0][:, 128:192], Wqkv[h1][:, 128:192]], axis=1
        ).astype(bf16)
        wv_c = wv_c.reshape(KD, 128, 128).transpose(1, 0, 2)
        bqk_c = np.stack(
            [
                np.concatenate([bqkv[h][0:64] * scale, bqkv[h][64:128]])[:, None]
                for h in (h0, h1)
            ]
        ).astype(np.float32)
        bv_c = np.concatenate([bqkv[h0][128:192], bqkv[h1][128:192]])[:, None].astype(
            np.float32
        )
        in_maps.append(
            {
                "xt": xtn,
                "wqk": np.ascontiguousarray(wqk_c),
                "wv": np.ascontiguousarray(wv_c),
                "bqk": np.ascontiguousarray(bqk_c),
                "bv": np.ascontiguousarray(bv_c),
                "wo": wo,
                "mask": mask,
                "ident": ident,
            }
        )
    return in_maps


def run_cores(in_maps, trace=False, trace_kwargs=None):
    nc = _get_nc()
    kwargs = {}
    if trace:
        kwargs["trace"] = True
        if trace_kwargs:
            kwargs["trace_kwargs"] = trace_kwargs
    return bass_utils.run_bass_kernel_spmd(
        nc, in_maps, core_ids=list(range(N_CORES)), **kwargs
    )


def assemble(results, bo):
    """Reassemble core outputs (interleaved token-tile mapping) into [B,S,D]."""
    full = np.empty((NT, D), np.float32)
    for c in range(N_CORES):
        o = results[c]["out"]
        for b in range(B):
            for pos in range(2):
                t = c + 8 * pos  # token tile within batch
                dst = b * S + t * 128
                full[dst : dst + 128] = o[(2 * b + pos) * 128 : (2 * b + pos + 1) * 128]
    full += bo[None, :]
    return full.reshape(B, S, D)


def kernel(x, Wqkv, bqkv, Wo, bo):
    x = np.asarray(x, dtype=np.float32)
    Wqkv = np.asarray(Wqkv, dtype=np.float32)
    bqkv = np.asarray(bqkv, dtype=np.float32)
    Wo = np.asarray(Wo, dtype=np.float32)
    bo = np.asarray(bo, dtype=np.float32)

    in_maps = make_in_maps(x, Wqkv, bqkv, Wo)
    res = run_cores(in_maps)
    return assemble(res.results, bo)
